# revision 45
# baseline (speedup 1.0000x reference)
"""Trainium2 Bass kernel: causal multi-head attention with interleaved RoPE.

Problem shapes (hardcoded): x [2, 2048, 1024], 16 heads of dk=64.
Sharding: 8 cores = 2 batches x 4 head-groups (4 heads each). Each core
computes its head-slice Q/K/V projections, RoPE, causal attention, and a
partial output through its Wo row-slice; the host sums the 4 partials per
batch and adds bo.

All matmul operands are fp16 (1 col/cycle at the full 2.4GHz PE clock;
fp32r is SBUF-bandwidth limited to ~1.3GHz effective). PSUM accumulation
stays fp32. End-to-end rel err ~6e-4 (budget 2e-2).

RoPE trick: attention scores are invariant to any permutation of the dk
axis applied to both Q and K, so the Wq/Wk columns are permuted on the host
into a "quadrant half-split" layout where each rotation pair partner sits
exactly 16 partitions away inside the same 32-partition quadrant. The DVE
stream_shuffle (a per-quadrant 32-way permute) then produces the swapped
operand, and RoPE becomes: rot = q * cosT + shuffle(q) * sinT with
host-precomputed tables (sinT carries the sign).

Phase order per PE queue: proj0, proj1, attn0, proj2, wo0, attn1, proj3,
attn2, wo1, attn3, wo2, wo3 — keeps the PE stream dependency-slack ahead
of the DVE/ACT producers (rope, exp, normalize) feeding it; each head's
normalize broadcast is deferred one head so the DVE reciprocal latency
hides behind the next head's matmuls.
"""

import os
from contextlib import ExitStack

import numpy as np

import concourse.bass as bass
import concourse.mybir as mybir
import concourse.tile as tile

B, S, D, H = 2, 2048, 1024, 16
DK = D // H  # 64
HG = 4  # heads per core
NCOLS = HG * DK  # 256 columns of the projection per core
THETA = 10000.0
SCALE = 1.0 / float(np.sqrt(DK))
N_CORES = 8

F32 = mybir.dt.float32
F32R = mybir.dt.float32r
F16 = mybir.dt.float16
MMDT = F16


def to_f16(a):
    return np.ascontiguousarray(np.asarray(a, dtype=np.float32).astype(np.float16))


# ---------------------------------------------------------------------------
# host-side prep
# ---------------------------------------------------------------------------

def _rope_perm():
    """Within-head column permutation pi: new row r -> original dk index."""
    perm = np.empty(DK, dtype=np.int64)
    for r in range(DK):
        q, m = divmod(r, 32)
        if m < 16:
            perm[r] = 2 * (16 * q + m)
        else:
            perm[r] = 2 * (16 * q + m - 16) + 1
    return perm


_PERM = _rope_perm()
SHUF_MASK = list(range(16, 32)) + list(range(16))  # swap 16-halves per quadrant


def _rope_tables(pos):
    """cosT/sinT [128, S] fp32 for the permuted layout. pos: [S] int."""
    inv_freq = (np.float32(THETA) ** (-(np.arange(0, DK, 2, dtype=np.float32) / np.float32(DK))))  # [32]
    ang = pos.astype(np.float32)[:, None] * inv_freq[None, :]  # [S, 32]
    cos = np.cos(ang)  # [S, 32]
    sin = np.sin(ang)
    cosT = np.empty((128, S), dtype=np.float32)
    sinT = np.empty((128, S), dtype=np.float32)
    for p in range(128):
        r = p % DK
        q, m = divmod(r, 32)
        if m < 16:
            i = 16 * q + m
            sgn = -1.0
        else:
            i = 16 * q + m - 16
            sgn = 1.0
        cosT[p] = cos[:, i]
        sinT[p] = np.float32(sgn) * sin[:, i]
    return cosT, sinT


def make_core_inputs(x, token_position, Wq, bq, Wk, bk, Wv, bv, Wo, bo):
    """Build the 8 per-core input maps."""
    x = np.asarray(x, dtype=np.float32)
    token_position = np.asarray(token_position)
    Wq, Wk, Wv, Wo = (np.asarray(w, dtype=np.float32) for w in (Wq, Wk, Wv, Wo))
    bq, bk, bv = (np.asarray(b_, dtype=np.float32) for b_ in (bq, bk, bv))

    in_maps = []
    tables = {}
    for c in range(N_CORES):
        b, hg = divmod(c, HG)
        heads = range(HG * hg, HG * hg + HG)
        # permuted q/k column indices for this core's heads
        cols_qk = np.concatenate([DK * h + _PERM for h in heads])
        cols_v = np.arange(NCOLS * hg, NCOLS * hg + NCOLS)
        if b not in tables:
            tables[b] = _rope_tables(np.asarray(token_position[b]))
        cosT, sinT = tables[b]
        wo_rows = Wo[cols_v, :]  # [256, 1024] (head-major rows)
        # pair-packed Wo: lane l of pair p holds Wo row of head 2p + l//64,
        # dk l%64 -- so one K=128 matmul contracts a full head pair.
        wo_packed = np.empty((128, 2, D), dtype=np.float32)
        for l_ in range(128):
            for p in range(2):
                h = 2 * p + l_ // 64
                wo_packed[l_, p, :] = wo_rows[DK * h + (l_ % 64), :]
        in_maps.append({
            "xT": to_f16(x[b].T),                               # [1024, 2048]
            "wq": to_f16(Wq[:, cols_qk]),                       # [1024, 256]
            "wk": to_f16(Wk[:, cols_qk]),
            "wv": to_f16(Wv[:, cols_v]),
            "wo": to_f16(wo_packed),                            # [128, 2, 1024]
            "bq": to_f16(bq[cols_qk][None, :]),                 # [1, 256]
            "bk": to_f16(bk[cols_qk][None, :]),
            "bv": to_f16(bv[cols_v][None, :]),
            "ones_row": to_f16(np.ones((1, 512), np.float32)),
            "onesc": to_f16(np.ones((128, DK), np.float32)),
            "cosT": cosT,
            "sinT": sinT,
        })
    return in_maps


# ---------------------------------------------------------------------------
# device program
# ---------------------------------------------------------------------------

def build_program(with_bias=False):
    from concourse import bacc, library_config
    nc = bacc.Bacc("TRN2", debug=False)

    xT = nc.declare_dram_parameter("xT", [D, S], MMDT, isOutput=False).ap()
    wq = nc.declare_dram_parameter("wq", [D, NCOLS], MMDT, isOutput=False).ap()
    wk = nc.declare_dram_parameter("wk", [D, NCOLS], MMDT, isOutput=False).ap()
    wv = nc.declare_dram_parameter("wv", [D, NCOLS], MMDT, isOutput=False).ap()
    wo = nc.declare_dram_parameter("wo", [128, 2, D], MMDT, isOutput=False).ap()
    bq = nc.declare_dram_parameter("bq", [1, NCOLS], MMDT, isOutput=False).ap()
    bk = nc.declare_dram_parameter("bk", [1, NCOLS], MMDT, isOutput=False).ap()
    bv = nc.declare_dram_parameter("bv", [1, NCOLS], MMDT, isOutput=False).ap()
    ones_row_d = nc.declare_dram_parameter("ones_row", [1, 512], MMDT, isOutput=False).ap()
    onesc_d = nc.declare_dram_parameter("onesc", [128, DK], MMDT, isOutput=False).ap()
    cosT = nc.declare_dram_parameter("cosT", [128, S], F32, isOutput=False).ap()
    sinT = nc.declare_dram_parameter("sinT", [128, S], F32, isOutput=False).ap()
    out = nc.declare_dram_parameter("out", [S, D], F16, isOutput=True).ap()

    SB = 512            # sq block width
    NSB = S // SB       # 4
    NST = S // 128      # 16 key tiles / V tiles
    NDC = D // 128      # 8 contraction chunks
    GW = 2              # key tiles per score-psum group
    AUGW = DK + 8       # V head stride (72): 16B-aligned fp16 lhsT starts

    with tile.TileContext(nc) as tc, ExitStack() as ctx:
        nc.gpsimd.load_library(library_config.proxy)
        const = ctx.enter_context(tc.tile_pool(name="const", bufs=1))
        sbig = ctx.enter_context(tc.tile_pool(name="sbig", bufs=1))
        xts = ctx.enter_context(tc.tile_pool(name="xts", bufs=8))
        rtmp = ctx.enter_context(tc.tile_pool(name="rtmp", bufs=2))
        epool = ctx.enter_context(tc.tile_pool(name="epool", bufs=3))
        npool = ctx.enter_context(tc.tile_pool(name="npool", bufs=5))
        opool = ctx.enter_context(tc.tile_pool(name="opool", bufs=4))
        mm_ps = ctx.enter_context(tc.tile_pool(name="mm_ps", bufs=2, space="PSUM"))
        sc_ps = ctx.enter_context(tc.tile_pool(name="sc_ps", bufs=2, space="PSUM"))
        pv_ps = ctx.enter_context(tc.tile_pool(name="pv_ps", bufs=2, space="PSUM"))

        # --- constants / weights resident in SBUF (per-dc tiles: finer deps,
        # so the first projection matmuls start early)
        wq_sb = [const.tile([128, NCOLS], MMDT, tag=f"wq{dc}", name=f"wq{dc}")
                 for dc in range(NDC)]
        wk_sb = [const.tile([128, NCOLS], MMDT, tag=f"wk{dc}", name=f"wk{dc}")
                 for dc in range(NDC)]
        wv_sb = [const.tile([128, NCOLS], MMDT, tag=f"wv{dc}", name=f"wv{dc}")
                 for dc in range(NDC)]
        # DMA priority: wq/wk chunks feed the very first matmuls; xt chunks
        # for sb=0 are emitted inside proj(0); tables/wv/wo right after them.
        cos_sb = const.tile([128, S], F32, tag="cos")
        sin_sb = const.tile([128, S], F32, tag="sin")
        wo_sb = const.tile([128, 2, D], MMDT, tag="wo")

        def late_dmas():
            # wk on the sync queue right behind the interleaved wq/xt pairs;
            # big tables ride the (idle-at-start) DVE trigger queue in
            # parallel so rope/V inputs land early without delaying xt.
            for dc in range(NDC):
                nc.sync.dma_start(wk_sb[dc][:], wk[128 * dc:128 * dc + 128, :])
            nc.scalar.dma_start(cos_sb[:], cosT)
            nc.scalar.dma_start(sin_sb[:], sinT)
            for dc in range(NDC):
                nc.scalar.dma_start(wv_sb[dc][:], wv[128 * dc:128 * dc + 128, :])
            nc.scalar.dma_start(wo_sb[:], wo)
        if with_bias:
            bq_sb = const.tile([1, NCOLS], MMDT, tag="bq")
            bk_sb = const.tile([1, NCOLS], MMDT, tag="bk")
            bv_sb = const.tile([1, NCOLS], MMDT, tag="bv")
            nc.sync.dma_start(bq_sb[:], bq)
            nc.sync.dma_start(bk_sb[:], bk)
            nc.sync.dma_start(bv_sb[:], bv)
        ones_row = const.tile([1, SB], MMDT, tag="ones_row")
        nc.sync.dma_start(ones_row[:], ones_row_d)
        onesc_sb = const.tile([128, DK], MMDT, tag="onesc")
        nc.sync.dma_start(onesc_sb[:], onesc_d)

        # Q^T per (chunk, sq-block): chunk c holds heads {2c, 2c+1}
        qt = [[sbig.tile([128, SB], MMDT, tag=f"qt{c}_{sb}", name=f"qt{c}_{sb}")
               for sb in range(NSB)] for c in range(2)]
        # per-head K^T, zero-padded to 128 partitions (head data on its chunk
        # rows, the complementary 64 rows zeroed)
        kth = [[sbig.tile([128, SB], MMDT, tag=f"kh{h}_{sb}", name=f"kh{h}_{sb}")
                for sb in range(NSB)] for h in range(HG)]
        for h in range(HG):
            zrows = slice(DK, 128) if h % 2 == 0 else slice(0, DK)
            for sb in range(NSB):
                nc.vector.memset(kth[h][sb][zrows, :], 0.0)
        # V augmented with a ones column per head, per key tile.
        vaug = [sbig.tile([128, HG * AUGW], MMDT, tag=f"va{st}", name=f"va{st}")
                for st in range(NST)]
        # normalized O^T per (head-pair, sq-block): lanes 0:64 = even head's
        # dk, 64:128 = odd head's dk -- Wo contracts a dense K=128 per pair.
        ot = [[sbig.tile([128, SB], MMDT, tag=f"ot{p}_{j}", name=f"ot{p}_{j}")
               for j in range(NSB)] for p in range(2)]

        ncopy = [0]

        def out_copy(dst, src):
            k = ncopy[0] % 2
            ncopy[0] += 1
            if k == 0:
                nc.vector.tensor_copy(dst, src)
            else:
                nc.scalar.copy(dst, src)

        def proj_units(sb, post_xt=None):
            """Emit the xt DMAs now; return 8 unit-closures (4 QK rope
            units + 4 V units) to be interleaved into the attention head
            loop so attention's DVE recips never queue behind rope work."""
            ss = slice(SB * sb, SB * sb + SB)
            xt_t = []
            for dc in range(NDC):
                if sb == 0:
                    # first phase: pair each wq chunk with its xt chunk so
                    # matmul dc can start as soon as pair dc lands
                    nc.sync.dma_start(wq_sb[dc][:], wq[128 * dc:128 * dc + 128, :])
                t = xts.tile([128, SB], MMDT, tag="xt", name=f"xt{sb}_{dc}")
                nc.sync.dma_start(t[:], xT[128 * dc:128 * dc + 128, ss])
                xt_t.append(t)
            if post_xt is not None:
                post_xt()

            def qk_unit(c, bname):
                ncol = slice(128 * c, 128 * c + 128)
                w_sb = wq_sb if bname == "bq" else wk_sb
                ps = mm_ps.tile([128, SB], F32, tag="mm")
                for dc in range(NDC):
                    nc.tensor.matmul(ps[:], w_sb[dc][:, ncol], xt_t[dc][:],
                                     start=(dc == 0),
                                     stop=(dc == NDC - 1 and not with_bias))
                if with_bias:
                    b_sb = bq_sb if bname == "bq" else bk_sb
                    nc.tensor.matmul(ps[:], b_sb[0:1, ncol], ones_row[0:1, :],
                                     start=False, stop=True)
                # rope: dst = ps*cos + shuffle(ps)*sin
                t_cos = rtmp.tile([128, SB], F32, tag="rc")
                nc.vector.tensor_mul(t_cos[:], ps[:], cos_sb[:, ss])
                t_shuf = rtmp.tile([128, SB], F32, tag="rs")
                nc.vector.stream_shuffle(t_shuf[:], ps[:], SHUF_MASK)
                t_sin = rtmp.tile([128, SB], F32, tag="rm")
                nc.gpsimd.tensor_mul(t_sin[:], t_shuf[:], sin_sb[:, ss])
                if bname == "bq":
                    nc.gpsimd.tensor_add(qt[c][sb][:], t_cos[:], t_sin[:])
                else:
                    nc.gpsimd.tensor_add(kth[2 * c][sb][0:DK, :],
                                         t_cos[0:DK, :], t_sin[0:DK, :])
                    nc.gpsimd.tensor_add(kth[2 * c + 1][sb][DK:128, :],
                                         t_cos[DK:128, :], t_sin[DK:128, :])

            def v_unit(st4):
                st = (SB // 128) * sb + st4
                ps = mm_ps.tile([128, SB], F32, tag="mm")
                for dc in range(NDC):
                    nc.tensor.matmul(ps[:, 0:NCOLS],
                                     xt_t[dc][:, 128 * st4:128 * st4 + 128],
                                     wv_sb[dc][:],
                                     start=(dc == 0),
                                     stop=(dc == NDC - 1 and not with_bias))
                if with_bias:
                    nc.tensor.matmul(ps[:, 0:NCOLS], ones_row[0:1, 0:128],
                                     bv_sb[0:1, :], start=False, stop=True)
                va = vaug[st][:].rearrange("p (h e) -> p h e", h=HG)
                nc.vector.tensor_copy(va[:, :, 0:DK],
                                      ps[:, 0:NCOLS].rearrange("p (h k) -> p h k", h=HG))
                nc.vector.tensor_copy(va[:, :, DK], onesc_sb[:, 0:HG])

            units = [lambda c=c, b=b: qk_unit(c, b)
                     for c in range(2) for b in ("bq", "bk")]
            units += [lambda st4=st4: v_unit(st4) for st4 in range(SB // 128)]
            return units

        def proj(sb, post_xt=None):
            finish_norm()
            for u in proj_units(sb, post_xt=post_xt):
                u()

        pend_norm = []

        def finish_one_norm():
            jj, h, pv16, rec16 = pend_norm.pop(0)
            p, u = divmod(h, 2)
            bcp = mm_ps.tile([128, SB], F32, tag="mm")
            nc.tensor.matmul(bcp[0:DK, :], onesc_sb[64:65, :],
                             rec16[DK:DK + 1, :],
                             start=True, stop=True)
            bc = npool.tile([128, SB], MMDT, tag="bc")
            nc.scalar.copy(bc[0:DK, :], bcp[0:DK, :])
            # all-SBUF fp16 multiply: DVE 2x/4x fast path
            nc.vector.tensor_mul(ot[p][jj][DK * u:DK * u + DK, :],
                                 pv16[0:DK, :], bc[0:DK, :])

        def finish_norm():
            while pend_norm:
                finish_one_norm()

        def attn(j, units=()):
            # S^T layout: psum group = GW key tiles x one sq block; exp on ACT
            # over the causally-valid column ranges only; PV accumulates
            # (V | ones) so row 64 is the softmax denominator. `units` are
            # next-projection closures interleaved two-per-head so their rope
            # work lands *behind* this phase's recips in the DVE queue.
            units = list(units)
            finish_norm()
            for h in range(HG):
                c, half = divmod(h, 2)
                pv = pv_ps.tile([128, SB], F32, tag="pv")
                ngrp = (4 * j + 4) // GW
                for g in range(ngrp):
                    sc = sc_ps.tile([128, GW * SB], F32, tag="sc")
                    # lo[t]: first causally-valid query column for key tile
                    # GW*g + t; scores/exp/PV all skip cols below it.
                    los = [min(max(128 * (GW * g + t - 4 * j), 0), SB)
                           for t in range(GW)]
                    for t in range(GW):
                        i = GW * g + t
                        lo = los[t]
                        if lo >= SB:
                            continue
                        nc.tensor.matmul(
                            sc[:, SB * t + lo:SB * t + SB],
                            kth[h][i // 4][:, 128 * (i % 4):128 * (i % 4) + 128],
                            qt[c][j][:, lo:SB],
                            start=True, stop=True)
                    e = epool.tile([128, GW * SB], MMDT, tag="e")
                    diag = GW * g + GW - 4 * j > 0
                    if not diag:
                        # one big exp call: ACT per-call overhead ~0.2us
                        nc.scalar.activation(e[:], sc[:],
                                             mybir.ActivationFunctionType.Exp,
                                             scale=SCALE)
                    else:
                        for t in range(GW):
                            lo = los[t]
                            if lo > 0:
                                # zero the causally-dead prefix: PV streams
                                # the full e width (uniform psum region)
                                nc.gpsimd.memset(e[:, SB * t:SB * t + lo], 0.0)
                            nc.scalar.activation(
                                e[:, SB * t + lo:SB * t + SB],
                                sc[:, SB * t + lo:SB * t + SB],
                                mybir.ActivationFunctionType.Exp,
                                scale=SCALE)
                            if 128 * (GW * g + t - 4 * j) >= 0:
                                # triangle band: zero e where query < key
                                band = e[:, SB * t + lo:SB * t + lo + 128]
                                nc.gpsimd.affine_select(
                                    out=band.rearrange("p (o f) -> p o f", o=1),
                                    in_=band.rearrange("p (o f) -> p o f", o=1),
                                    compare_op=mybir.AluOpType.is_ge,
                                    fill=0.0, base=0,
                                    pattern=[[-128, 1], [1, 128]],
                                    channel_multiplier=-1)
                    for t in range(GW):
                        i = GW * g + t
                        lhs = vaug[i][:].rearrange("p (h e) -> p h e", h=HG)[:, h, 0:DK + 1]
                        nc.tensor.matmul(
                            pv[0:DK + 1, :], lhs, e[:, SB * t:SB * t + SB],
                            start=(g == 0 and t == 0),
                            stop=(g == ngrp - 1 and t == GW - 1))
                # normalize: copy pv to fp16 SBUF right away (frees the
                # psum ring slot), DVE reciprocal (table-free; ACT recip
                # would thrash activation-table loads), then defer the PE
                # broadcast + scale TWO heads so the reciprocal latency and
                # DVE backlog always hide behind matmul streams.
                pv16 = npool.tile([128, SB], MMDT, tag="pv16")
                nc.vector.tensor_copy(pv16[0:DK + 1, :], pv[0:DK + 1, :])
                rec16 = npool.tile([128, SB], MMDT, tag="rec16")
                with nc.allow_low_precision(reason="denominator recip in fp16"):
                    nc.vector.reciprocal(rec16[DK:DK + 1, :], pv[DK:DK + 1, :])
                if len(pend_norm) >= 2:
                    finish_one_norm()
                pend_norm.append((j, h, pv16, rec16))
                for u in units[2 * h:2 * h + 2]:
                    u()

        def wo_phase(jb):
            finish_norm()
            for st4 in range(4):
                st = 4 * jb + st4
                rq = slice(128 * st4, 128 * st4 + 128)
                for dc in range(2):
                    cols = slice(SB * dc, SB * dc + SB)
                    ps = mm_ps.tile([128, SB], F32, tag="mm")
                    for p in range(2):
                        nc.tensor.matmul(ps[:], ot[p][jb][:, rq], wo_sb[:, p, cols],
                                         start=(p == 0), stop=(p == 1))
                    o_sb = opool.tile([128, SB], F16, tag="osb")
                    out_copy(o_sb[:], ps[:])
                    nc.sync.dma_start(out[128 * st:128 * st + 128, cols], o_sb[:])

        # phase schedule: PE stream stays ~2 phases ahead of its producers
        proj(0, post_xt=late_dmas)
        proj(1)
        attn(0)
        proj(2)
        wo_phase(0)
        attn(1)
        proj(3)
        attn(2)
        wo_phase(1)
        attn(3)
        wo_phase(2)
        wo_phase(3)

    nc.compile()
    return nc


_CACHED_NC = {}


def _get_program(with_bias=False):
    if with_bias not in _CACHED_NC:
        _CACHED_NC[with_bias] = build_program(with_bias=with_bias)
    return _CACHED_NC[with_bias]


# ---------------------------------------------------------------------------
# entry point
# ---------------------------------------------------------------------------

def kernel(x, token_position, Wq, bq, Wk, bk, Wv, bv, Wo, bo, _results=None):
    from concourse.bass_utils import run_bass_kernel_spmd

    in_maps = make_core_inputs(x, token_position, Wq, bq, Wk, bk, Wv, bv, Wo, bo)
    if _results is None:
        with_bias = any(float(np.abs(np.asarray(v)).max()) != 0.0
                        for v in (bq, bk, bv))
        nc = _get_program(with_bias=with_bias)
        res = run_bass_kernel_spmd(nc, in_maps, list(range(N_CORES)))
        _results = [res.results[i]["out"] for i in range(N_CORES)]
    bo = np.asarray(bo, dtype=np.float32)
    out = np.empty((B, S, D), dtype=np.float32)
    for b in range(B):
        acc = np.asarray(_results[HG * b], dtype=np.float32)
        for hg in range(1, HG):
            acc = acc + np.asarray(_results[HG * b + hg], dtype=np.float32)
        out[b] = acc + bo[None, :]
    return out


# revision 46
# speedup vs baseline: 1.0100x; 1.0100x over previous
"""Trainium2 Bass kernel: causal multi-head attention with interleaved RoPE.

Problem shapes (hardcoded): x [2, 2048, 1024], 16 heads of dk=64.
Sharding: 8 cores = 2 batches x 4 head-groups (4 heads each). Each core
computes its head-slice Q/K/V projections, RoPE, causal attention, and a
partial output through its Wo row-slice; the host sums the 4 partials per
batch and adds bo.

All matmul operands are fp16 (1 col/cycle at the full 2.4GHz PE clock;
fp32r is SBUF-bandwidth limited to ~1.3GHz effective). PSUM accumulation
stays fp32. End-to-end rel err ~6e-4 (budget 2e-2).

RoPE trick: attention scores are invariant to any permutation of the dk
axis applied to both Q and K, so the Wq/Wk columns are permuted on the host
into a "quadrant half-split" layout where each rotation pair partner sits
exactly 16 partitions away inside the same 32-partition quadrant. The DVE
stream_shuffle (a per-quadrant 32-way permute) then produces the swapped
operand, and RoPE becomes: rot = q * cosT + shuffle(q) * sinT with
host-precomputed tables (sinT carries the sign).

Phase order per PE queue: proj0, proj1, attn0, proj2, wo0, attn1, proj3,
attn2, wo1, attn3, wo2, wo3 — keeps the PE stream dependency-slack ahead
of the DVE/ACT producers (rope, exp, normalize) feeding it; each head's
normalize broadcast is deferred one head so the DVE reciprocal latency
hides behind the next head's matmuls.
"""

import os
from contextlib import ExitStack

import numpy as np

import concourse.bass as bass
import concourse.mybir as mybir
import concourse.tile as tile

B, S, D, H = 2, 2048, 1024, 16
DK = D // H  # 64
HG = 4  # heads per core
NCOLS = HG * DK  # 256 columns of the projection per core
THETA = 10000.0
SCALE = 1.0 / float(np.sqrt(DK))
N_CORES = 8

F32 = mybir.dt.float32
F32R = mybir.dt.float32r
F16 = mybir.dt.float16
MMDT = F16


def to_f16(a):
    return np.ascontiguousarray(np.asarray(a, dtype=np.float32).astype(np.float16))


# ---------------------------------------------------------------------------
# host-side prep
# ---------------------------------------------------------------------------

def _rope_perm():
    """Within-head column permutation pi: new row r -> original dk index."""
    perm = np.empty(DK, dtype=np.int64)
    for r in range(DK):
        q, m = divmod(r, 32)
        if m < 16:
            perm[r] = 2 * (16 * q + m)
        else:
            perm[r] = 2 * (16 * q + m - 16) + 1
    return perm


_PERM = _rope_perm()
SHUF_MASK = list(range(16, 32)) + list(range(16))  # swap 16-halves per quadrant


def _rope_tables(pos):
    """cosT/sinT [128, S] fp32 for the permuted layout. pos: [S] int."""
    inv_freq = (np.float32(THETA) ** (-(np.arange(0, DK, 2, dtype=np.float32) / np.float32(DK))))  # [32]
    ang = pos.astype(np.float32)[:, None] * inv_freq[None, :]  # [S, 32]
    cos = np.cos(ang)  # [S, 32]
    sin = np.sin(ang)
    cosT = np.empty((128, S), dtype=np.float32)
    sinT = np.empty((128, S), dtype=np.float32)
    for p in range(128):
        r = p % DK
        q, m = divmod(r, 32)
        if m < 16:
            i = 16 * q + m
            sgn = -1.0
        else:
            i = 16 * q + m - 16
            sgn = 1.0
        cosT[p] = cos[:, i]
        sinT[p] = np.float32(sgn) * sin[:, i]
    return cosT, sinT


def make_core_inputs(x, token_position, Wq, bq, Wk, bk, Wv, bv, Wo, bo):
    """Build the 8 per-core input maps."""
    x = np.asarray(x, dtype=np.float32)
    token_position = np.asarray(token_position)
    Wq, Wk, Wv, Wo = (np.asarray(w, dtype=np.float32) for w in (Wq, Wk, Wv, Wo))
    bq, bk, bv = (np.asarray(b_, dtype=np.float32) for b_ in (bq, bk, bv))

    in_maps = []
    tables = {}
    for c in range(N_CORES):
        b, hg = divmod(c, HG)
        heads = range(HG * hg, HG * hg + HG)
        # permuted q/k column indices for this core's heads
        cols_qk = np.concatenate([DK * h + _PERM for h in heads])
        cols_v = np.arange(NCOLS * hg, NCOLS * hg + NCOLS)
        if b not in tables:
            tables[b] = _rope_tables(np.asarray(token_position[b]))
        cosT, sinT = tables[b]
        wo_rows = Wo[cols_v, :]  # [256, 1024] (head-major rows)
        # pair-packed Wo: lane l of pair p holds Wo row of head 2p + l//64,
        # dk l%64 -- so one K=128 matmul contracts a full head pair.
        wo_packed = np.empty((128, 2, D), dtype=np.float32)
        for l_ in range(128):
            for p in range(2):
                h = 2 * p + l_ // 64
                wo_packed[l_, p, :] = wo_rows[DK * h + (l_ % 64), :]
        in_maps.append({
            "xT": to_f16(x[b].T),                               # [1024, 2048]
            "wq": to_f16(Wq[:, cols_qk]),                       # [1024, 256]
            "wk": to_f16(Wk[:, cols_qk]),
            "wv": to_f16(Wv[:, cols_v]),
            "wo": to_f16(wo_packed),                            # [128, 2, 1024]
            "bq": to_f16(bq[cols_qk][None, :]),                 # [1, 256]
            "bk": to_f16(bk[cols_qk][None, :]),
            "bv": to_f16(bv[cols_v][None, :]),
            "ones_row": to_f16(np.ones((1, 512), np.float32)),
            "onesc": to_f16(np.ones((128, DK), np.float32)),
            "cosT": cosT,
            "sinT": sinT,
        })
    return in_maps


# ---------------------------------------------------------------------------
# device program
# ---------------------------------------------------------------------------

def build_program(with_bias=False):
    from concourse import bacc, library_config
    nc = bacc.Bacc("TRN2", debug=False)

    xT = nc.declare_dram_parameter("xT", [D, S], MMDT, isOutput=False).ap()
    wq = nc.declare_dram_parameter("wq", [D, NCOLS], MMDT, isOutput=False).ap()
    wk = nc.declare_dram_parameter("wk", [D, NCOLS], MMDT, isOutput=False).ap()
    wv = nc.declare_dram_parameter("wv", [D, NCOLS], MMDT, isOutput=False).ap()
    wo = nc.declare_dram_parameter("wo", [128, 2, D], MMDT, isOutput=False).ap()
    bq = nc.declare_dram_parameter("bq", [1, NCOLS], MMDT, isOutput=False).ap()
    bk = nc.declare_dram_parameter("bk", [1, NCOLS], MMDT, isOutput=False).ap()
    bv = nc.declare_dram_parameter("bv", [1, NCOLS], MMDT, isOutput=False).ap()
    ones_row_d = nc.declare_dram_parameter("ones_row", [1, 512], MMDT, isOutput=False).ap()
    onesc_d = nc.declare_dram_parameter("onesc", [128, DK], MMDT, isOutput=False).ap()
    cosT = nc.declare_dram_parameter("cosT", [128, S], F32, isOutput=False).ap()
    sinT = nc.declare_dram_parameter("sinT", [128, S], F32, isOutput=False).ap()
    out = nc.declare_dram_parameter("out", [S, D], F16, isOutput=True).ap()

    SB = 512            # sq block width
    NSB = S // SB       # 4
    NST = S // 128      # 16 key tiles / V tiles
    NDC = D // 128      # 8 contraction chunks
    GW = 2              # key tiles per score-psum group
    AUGW = DK + 8       # V head stride (72): 16B-aligned fp16 lhsT starts

    with tile.TileContext(nc) as tc, ExitStack() as ctx:
        nc.gpsimd.load_library(library_config.proxy)
        const = ctx.enter_context(tc.tile_pool(name="const", bufs=1))
        sbig = ctx.enter_context(tc.tile_pool(name="sbig", bufs=1))
        xts = ctx.enter_context(tc.tile_pool(name="xts", bufs=8))
        rtmp = ctx.enter_context(tc.tile_pool(name="rtmp", bufs=2))
        epool = ctx.enter_context(tc.tile_pool(name="epool", bufs=3))
        npool = ctx.enter_context(tc.tile_pool(name="npool", bufs=5))
        opool = ctx.enter_context(tc.tile_pool(name="opool", bufs=4))
        mm_ps = ctx.enter_context(tc.tile_pool(name="mm_ps", bufs=2, space="PSUM"))
        sc_ps = ctx.enter_context(tc.tile_pool(name="sc_ps", bufs=2, space="PSUM"))
        pv_ps = ctx.enter_context(tc.tile_pool(name="pv_ps", bufs=2, space="PSUM"))

        # --- constants / weights resident in SBUF (per-dc tiles: finer deps,
        # so the first projection matmuls start early)
        wq_sb = [const.tile([128, NCOLS], MMDT, tag=f"wq{dc}", name=f"wq{dc}")
                 for dc in range(NDC)]
        wk_sb = [const.tile([128, NCOLS], MMDT, tag=f"wk{dc}", name=f"wk{dc}")
                 for dc in range(NDC)]
        wv_sb = [const.tile([128, NCOLS], MMDT, tag=f"wv{dc}", name=f"wv{dc}")
                 for dc in range(NDC)]
        # DMA priority: wq/wk chunks feed the very first matmuls; xt chunks
        # for sb=0 are emitted inside proj(0); tables/wv/wo right after them.
        cos_sb = const.tile([128, S], F32, tag="cos")
        sin_sb = const.tile([128, S], F32, tag="sin")
        wo_sb = const.tile([128, 2, D], MMDT, tag="wo")

        def late_dmas():
            # wk on the sync queue right behind the interleaved wq/xt pairs;
            # big tables ride the (idle-at-start) DVE trigger queue in
            # parallel so rope/V inputs land early without delaying xt.
            for dc in range(NDC):
                nc.sync.dma_start(wk_sb[dc][:], wk[128 * dc:128 * dc + 128, :])
            nc.scalar.dma_start(cos_sb[:], cosT)
            nc.scalar.dma_start(sin_sb[:], sinT)
            for dc in range(NDC):
                nc.scalar.dma_start(wv_sb[dc][:], wv[128 * dc:128 * dc + 128, :])
            nc.scalar.dma_start(wo_sb[:], wo)
        if with_bias:
            bq_sb = const.tile([1, NCOLS], MMDT, tag="bq")
            bk_sb = const.tile([1, NCOLS], MMDT, tag="bk")
            bv_sb = const.tile([1, NCOLS], MMDT, tag="bv")
            nc.sync.dma_start(bq_sb[:], bq)
            nc.sync.dma_start(bk_sb[:], bk)
            nc.sync.dma_start(bv_sb[:], bv)
        ones_row = const.tile([1, SB], MMDT, tag="ones_row")
        nc.sync.dma_start(ones_row[:], ones_row_d)
        onesc_sb = const.tile([128, DK], MMDT, tag="onesc")
        nc.sync.dma_start(onesc_sb[:], onesc_d)

        # Q^T per (chunk, sq-block): chunk c holds heads {2c, 2c+1}
        qt = [[sbig.tile([128, SB], MMDT, tag=f"qt{c}_{sb}", name=f"qt{c}_{sb}")
               for sb in range(NSB)] for c in range(2)]
        # per-head K^T, zero-padded to 128 partitions (head data on its chunk
        # rows, the complementary 64 rows zeroed)
        kth = [[sbig.tile([128, SB], MMDT, tag=f"kh{h}_{sb}", name=f"kh{h}_{sb}")
                for sb in range(NSB)] for h in range(HG)]
        for h in range(HG):
            zrows = slice(DK, 128) if h % 2 == 0 else slice(0, DK)
            for sb in range(NSB):
                nc.vector.memset(kth[h][sb][zrows, :], 0.0)
        # V augmented with a ones column per head, per key tile.
        vaug = [sbig.tile([128, HG * AUGW], MMDT, tag=f"va{st}", name=f"va{st}")
                for st in range(NST)]
        # normalized O^T per (head-pair, sq-block): lanes 0:64 = even head's
        # dk, 64:128 = odd head's dk -- Wo contracts a dense K=128 per pair.
        ot = [[sbig.tile([128, SB], MMDT, tag=f"ot{p}_{j}", name=f"ot{p}_{j}")
               for j in range(NSB)] for p in range(2)]

        ncopy = [0]

        def out_copy(dst, src):
            k = ncopy[0] % 2
            ncopy[0] += 1
            if k == 0:
                nc.vector.tensor_copy(dst, src)
            else:
                nc.scalar.copy(dst, src)

        def proj_units(sb, post_xt=None):
            """Emit the xt DMAs now; return 8 unit-closures (4 QK rope
            units + 4 V units) to be interleaved into the attention head
            loop so attention's DVE recips never queue behind rope work."""
            ss = slice(SB * sb, SB * sb + SB)
            xt_t = []
            for dc in range(NDC):
                if sb == 0:
                    # first phase: pair each wq chunk with its xt chunk so
                    # matmul dc can start as soon as pair dc lands
                    nc.sync.dma_start(wq_sb[dc][:], wq[128 * dc:128 * dc + 128, :])
                t = xts.tile([128, SB], MMDT, tag="xt", name=f"xt{sb}_{dc}")
                nc.sync.dma_start(t[:], xT[128 * dc:128 * dc + 128, ss])
                xt_t.append(t)
            if post_xt is not None:
                post_xt()

            def qk_unit(c, bname):
                ncol = slice(128 * c, 128 * c + 128)
                w_sb = wq_sb if bname == "bq" else wk_sb
                ps = mm_ps.tile([128, SB], F32, tag="mm")
                for dc in range(NDC):
                    nc.tensor.matmul(ps[:], w_sb[dc][:, ncol], xt_t[dc][:],
                                     start=(dc == 0),
                                     stop=(dc == NDC - 1 and not with_bias))
                if with_bias:
                    b_sb = bq_sb if bname == "bq" else bk_sb
                    nc.tensor.matmul(ps[:], b_sb[0:1, ncol], ones_row[0:1, :],
                                     start=False, stop=True)
                # rope: dst = ps*cos + shuffle(ps)*sin
                t_cos = rtmp.tile([128, SB], F32, tag="rc")
                nc.vector.tensor_mul(t_cos[:], ps[:], cos_sb[:, ss])
                t_shuf = rtmp.tile([128, SB], F32, tag="rs")
                nc.vector.stream_shuffle(t_shuf[:], ps[:], SHUF_MASK)
                t_sin = rtmp.tile([128, SB], F32, tag="rm")
                nc.gpsimd.tensor_mul(t_sin[:], t_shuf[:], sin_sb[:, ss])
                if bname == "bq":
                    nc.gpsimd.tensor_add(qt[c][sb][:], t_cos[:], t_sin[:])
                else:
                    nc.gpsimd.tensor_add(kth[2 * c][sb][0:DK, :],
                                         t_cos[0:DK, :], t_sin[0:DK, :])
                    nc.gpsimd.tensor_add(kth[2 * c + 1][sb][DK:128, :],
                                         t_cos[DK:128, :], t_sin[DK:128, :])

            def v_unit(st4):
                st = (SB // 128) * sb + st4
                ps = mm_ps.tile([128, SB], F32, tag="mm")
                for dc in range(NDC):
                    nc.tensor.matmul(ps[:, 0:NCOLS],
                                     xt_t[dc][:, 128 * st4:128 * st4 + 128],
                                     wv_sb[dc][:],
                                     start=(dc == 0),
                                     stop=(dc == NDC - 1 and not with_bias))
                if with_bias:
                    nc.tensor.matmul(ps[:, 0:NCOLS], ones_row[0:1, 0:128],
                                     bv_sb[0:1, :], start=False, stop=True)
                va = vaug[st][:].rearrange("p (h e) -> p h e", h=HG)
                nc.vector.tensor_copy(va[:, :, 0:DK],
                                      ps[:, 0:NCOLS].rearrange("p (h k) -> p h k", h=HG))
                nc.vector.tensor_copy(va[:, :, DK], onesc_sb[:, 0:HG])

            units = [lambda c=c, b=b: qk_unit(c, b)
                     for c in range(2) for b in ("bq", "bk")]
            units += [lambda st4=st4: v_unit(st4) for st4 in range(SB // 128)]
            return units

        def proj(sb, post_xt=None):
            us = proj_units(sb, post_xt=post_xt)
            us[0]()
            finish_norm()
            for u in us[1:]:
                u()

        def raw_act(out_ap, in_ap, func, scale=1.0):
            eng = nc.scalar
            inputs = [eng.lower_ap(in_ap),
                      mybir.ImmediateValue(dtype=mybir.dt.float32, value=0.0),
                      mybir.ImmediateValue(dtype=mybir.dt.float32, value=scale),
                      mybir.ImmediateValue(dtype=mybir.dt.float32, value=0.0)]
            return eng.add_instruction(mybir.InstActivation(
                name=eng.bass.get_next_instruction_name(),
                func=func, ins=inputs, outs=[eng.lower_ap(out_ap)]))

        pend_norm = []

        def finish_one_norm():
            jj, h, pv16, rec16 = pend_norm.pop(0)
            p, u = divmod(h, 2)
            bcp = mm_ps.tile([128, SB], F32, tag="mm")
            nc.tensor.matmul(bcp[0:DK, :], onesc_sb[64:65, :],
                             rec16[DK:DK + 1, :],
                             start=True, stop=True)
            bc = npool.tile([128, SB], MMDT, tag="bc")
            nc.scalar.copy(bc[0:DK, :], bcp[0:DK, :])
            # all-SBUF fp16 multiply: DVE 2x/4x fast path
            nc.vector.tensor_mul(ot[p][jj][DK * u:DK * u + DK, :],
                                 pv16[0:DK, :], bc[0:DK, :])

        def finish_norm():
            while pend_norm:
                finish_one_norm()

        def attn(j, units=()):
            # S^T layout: psum group = GW key tiles x one sq block; exp on ACT
            # over the causally-valid column ranges only; PV accumulates
            # (V | ones) so row 64 is the softmax denominator. `units` are
            # next-projection closures interleaved two-per-head so their rope
            # work lands *behind* this phase's recips in the DVE queue.
            units = list(units)
            finish_norm()
            local_norms = []
            for h in range(HG):
                c, half = divmod(h, 2)
                pv = pv_ps.tile([128, SB], F32, tag="pv")
                ngrp = (4 * j + 4) // GW
                for g in range(ngrp):
                    sc = sc_ps.tile([128, GW * SB], F32, tag="sc")
                    # lo[t]: first causally-valid query column for key tile
                    # GW*g + t; scores/exp/PV all skip cols below it.
                    los = [min(max(128 * (GW * g + t - 4 * j), 0), SB)
                           for t in range(GW)]
                    for t in range(GW):
                        i = GW * g + t
                        lo = los[t]
                        if lo >= SB:
                            continue
                        nc.tensor.matmul(
                            sc[:, SB * t + lo:SB * t + SB],
                            kth[h][i // 4][:, 128 * (i % 4):128 * (i % 4) + 128],
                            qt[c][j][:, lo:SB],
                            start=True, stop=True)
                    e = epool.tile([128, GW * SB], MMDT, tag="e")
                    diag = GW * g + GW - 4 * j > 0
                    if not diag:
                        # one big exp call: ACT per-call overhead ~0.2us
                        nc.scalar.activation(e[:], sc[:],
                                             mybir.ActivationFunctionType.Exp,
                                             scale=SCALE)
                    else:
                        for t in range(GW):
                            lo = los[t]
                            if lo > 0:
                                # zero the causally-dead prefix: PV streams
                                # the full e width (uniform psum region)
                                nc.gpsimd.memset(e[:, SB * t:SB * t + lo], 0.0)
                            nc.scalar.activation(
                                e[:, SB * t + lo:SB * t + SB],
                                sc[:, SB * t + lo:SB * t + SB],
                                mybir.ActivationFunctionType.Exp,
                                scale=SCALE)
                            if 128 * (GW * g + t - 4 * j) >= 0:
                                # triangle band: zero e where query < key
                                band = e[:, SB * t + lo:SB * t + lo + 128]
                                nc.gpsimd.affine_select(
                                    out=band.rearrange("p (o f) -> p o f", o=1),
                                    in_=band.rearrange("p (o f) -> p o f", o=1),
                                    compare_op=mybir.AluOpType.is_ge,
                                    fill=0.0, base=0,
                                    pattern=[[-128, 1], [1, 128]],
                                    channel_multiplier=-1)
                    for t in range(GW):
                        i = GW * g + t
                        lhs = vaug[i][:].rearrange("p (h e) -> p h e", h=HG)[:, h, 0:DK + 1]
                        nc.tensor.matmul(
                            pv[0:DK + 1, :], lhs, e[:, SB * t:SB * t + SB],
                            start=(g == 0 and t == 0),
                            stop=(g == ngrp - 1 and t == GW - 1))
                # normalize part 1: copy pv to fp16 SBUF right away (frees
                # the psum ring slot; the mul later runs all-SBUF-fp16)
                pv16 = npool.tile([128, SB], MMDT, tag="pv16")
                nc.vector.tensor_copy(pv16[0:DK + 1, :], pv[0:DK + 1, :])
                local_norms.append((h, pv16))
            # normalize part 2: all 4 heads' reciprocals batched on ACT
            # (raw Reciprocal, ~1e-5 accurate) -> one activation-table load
            # here + one exp reload next phase, instead of per-head DVE
            # reciprocals whose 3.4us true latency the scheduler mis-prices
            # and turns into PE stalls. Broadcast+scale flush next phase.
            for h, pv16 in local_norms:
                rec16 = npool.tile([128, SB], MMDT, tag="rec16")
                raw_act(rec16[DK:DK + 1, :], pv16[DK:DK + 1, :],
                        mybir.ActivationFunctionType.Reciprocal)
                pend_norm.append((j, h, pv16, rec16))
                for u in units[2 * h:2 * h + 2]:
                    u()

        def wo_phase(jb):
            for st4 in range(4):
                if st4 == 1:
                    finish_norm()
                st = 4 * jb + st4
                rq = slice(128 * st4, 128 * st4 + 128)
                for dc in range(2):
                    cols = slice(SB * dc, SB * dc + SB)
                    ps = mm_ps.tile([128, SB], F32, tag="mm")
                    for p in range(2):
                        nc.tensor.matmul(ps[:], ot[p][jb][:, rq], wo_sb[:, p, cols],
                                         start=(p == 0), stop=(p == 1))
                    o_sb = opool.tile([128, SB], F16, tag="osb")
                    out_copy(o_sb[:], ps[:])
                    nc.sync.dma_start(out[128 * st:128 * st + 128, cols], o_sb[:])

        # phase schedule: PE stream stays ~2 phases ahead of its producers
        proj(0, post_xt=late_dmas)
        proj(1)
        attn(0)
        proj(2)
        wo_phase(0)
        attn(1)
        proj(3)
        attn(2)
        wo_phase(1)
        attn(3)
        wo_phase(2)
        wo_phase(3)

    nc.compile()
    return nc


_CACHED_NC = {}


def _get_program(with_bias=False):
    if with_bias not in _CACHED_NC:
        _CACHED_NC[with_bias] = build_program(with_bias=with_bias)
    return _CACHED_NC[with_bias]


# ---------------------------------------------------------------------------
# entry point
# ---------------------------------------------------------------------------

def kernel(x, token_position, Wq, bq, Wk, bk, Wv, bv, Wo, bo, _results=None):
    from concourse.bass_utils import run_bass_kernel_spmd

    in_maps = make_core_inputs(x, token_position, Wq, bq, Wk, bk, Wv, bv, Wo, bo)
    if _results is None:
        with_bias = any(float(np.abs(np.asarray(v)).max()) != 0.0
                        for v in (bq, bk, bv))
        nc = _get_program(with_bias=with_bias)
        res = run_bass_kernel_spmd(nc, in_maps, list(range(N_CORES)))
        _results = [res.results[i]["out"] for i in range(N_CORES)]
    bo = np.asarray(bo, dtype=np.float32)
    out = np.empty((B, S, D), dtype=np.float32)
    for b in range(B):
        acc = np.asarray(_results[HG * b], dtype=np.float32)
        for hg in range(1, HG):
            acc = acc + np.asarray(_results[HG * b + hg], dtype=np.float32)
        out[b] = acc + bo[None, :]
    return out


# revision 47
# speedup vs baseline: 1.0465x; 1.0361x over previous
"""Trainium2 Bass kernel: causal multi-head attention with interleaved RoPE.

Problem shapes (hardcoded): x [2, 2048, 1024], 16 heads of dk=64.
Sharding: 8 cores = 2 batches x 4 head-groups (4 heads each). Each core
computes its head-slice Q/K/V projections, RoPE, causal attention, and a
partial output through its Wo row-slice; the host sums the 4 partials per
batch and adds bo.

All matmul operands are fp16 (1 col/cycle at the full 2.4GHz PE clock;
fp32r is SBUF-bandwidth limited to ~1.3GHz effective). PSUM accumulation
stays fp32. End-to-end rel err ~6e-4 (budget 2e-2).

RoPE trick: attention scores are invariant to any permutation of the dk
axis applied to both Q and K, so the Wq/Wk columns are permuted on the host
into a "quadrant half-split" layout where each rotation pair partner sits
exactly 16 partitions away inside the same 32-partition quadrant. The DVE
stream_shuffle (a per-quadrant 32-way permute) then produces the swapped
operand, and RoPE becomes: rot = q * cosT + shuffle(q) * sinT with
host-precomputed tables (sinT carries the sign).

Phase order per PE queue: proj0, proj1, attn0, proj2, wo0, attn1, proj3,
attn2, wo1, attn3, wo2, wo3 — keeps the PE stream dependency-slack ahead
of the DVE/ACT producers (rope, exp, normalize) feeding it; each head's
normalize broadcast is deferred one head so the DVE reciprocal latency
hides behind the next head's matmuls.
"""

import os
from contextlib import ExitStack

import numpy as np

import concourse.bass as bass
import concourse.mybir as mybir
import concourse.tile as tile

B, S, D, H = 2, 2048, 1024, 16
DK = D // H  # 64
HG = 4  # heads per core
NCOLS = HG * DK  # 256 columns of the projection per core
THETA = 10000.0
SCALE = 1.0 / float(np.sqrt(DK))
N_CORES = 8

F32 = mybir.dt.float32
F32R = mybir.dt.float32r
F16 = mybir.dt.float16
MMDT = F16


def to_f16(a):
    return np.ascontiguousarray(np.asarray(a, dtype=np.float32).astype(np.float16))


# ---------------------------------------------------------------------------
# host-side prep
# ---------------------------------------------------------------------------

def _rope_perm():
    """Within-head column permutation pi: new row r -> original dk index."""
    perm = np.empty(DK, dtype=np.int64)
    for r in range(DK):
        q, m = divmod(r, 32)
        if m < 16:
            perm[r] = 2 * (16 * q + m)
        else:
            perm[r] = 2 * (16 * q + m - 16) + 1
    return perm


_PERM = _rope_perm()
SHUF_MASK = list(range(16, 32)) + list(range(16))  # swap 16-halves per quadrant


def _rope_tables(pos):
    """cosT/sinT [128, S] fp32 for the permuted layout. pos: [S] int."""
    inv_freq = (np.float32(THETA) ** (-(np.arange(0, DK, 2, dtype=np.float32) / np.float32(DK))))  # [32]
    ang = pos.astype(np.float32)[:, None] * inv_freq[None, :]  # [S, 32]
    cos = np.cos(ang)  # [S, 32]
    sin = np.sin(ang)
    cosT = np.empty((128, S), dtype=np.float32)
    sinT = np.empty((128, S), dtype=np.float32)
    for p in range(128):
        r = p % DK
        q, m = divmod(r, 32)
        if m < 16:
            i = 16 * q + m
            sgn = -1.0
        else:
            i = 16 * q + m - 16
            sgn = 1.0
        cosT[p] = cos[:, i]
        sinT[p] = np.float32(sgn) * sin[:, i]
    return cosT, sinT


def make_core_inputs(x, token_position, Wq, bq, Wk, bk, Wv, bv, Wo, bo):
    """Build the 8 per-core input maps."""
    x = np.asarray(x, dtype=np.float32)
    token_position = np.asarray(token_position)
    Wq, Wk, Wv, Wo = (np.asarray(w, dtype=np.float32) for w in (Wq, Wk, Wv, Wo))
    bq, bk, bv = (np.asarray(b_, dtype=np.float32) for b_ in (bq, bk, bv))

    in_maps = []
    tables = {}
    for c in range(N_CORES):
        b, hg = divmod(c, HG)
        heads = range(HG * hg, HG * hg + HG)
        # permuted q/k column indices for this core's heads
        cols_qk = np.concatenate([DK * h + _PERM for h in heads])
        cols_v = np.arange(NCOLS * hg, NCOLS * hg + NCOLS)
        if b not in tables:
            tables[b] = _rope_tables(np.asarray(token_position[b]))
        cosT, sinT = tables[b]
        wo_rows = Wo[cols_v, :]  # [256, 1024] (head-major rows)
        # pair-packed Wo: lane l of pair p holds Wo row of head 2p + l//64,
        # dk l%64 -- so one K=128 matmul contracts a full head pair.
        wo_packed = np.empty((128, 2, D), dtype=np.float32)
        for l_ in range(128):
            for p in range(2):
                h = 2 * p + l_ // 64
                wo_packed[l_, p, :] = wo_rows[DK * h + (l_ % 64), :]
        in_maps.append({
            "xT": to_f16(x[b].T),                               # [1024, 2048]
            "wq": to_f16(Wq[:, cols_qk]),                       # [1024, 256]
            "wk": to_f16(Wk[:, cols_qk]),
            "wv": to_f16(Wv[:, cols_v]),
            "wo": to_f16(wo_packed),                            # [128, 2, 1024]
            "bq": to_f16(bq[cols_qk][None, :]),                 # [1, 256]
            "bk": to_f16(bk[cols_qk][None, :]),
            "bv": to_f16(bv[cols_v][None, :]),
            "ones_row": to_f16(np.ones((1, 512), np.float32)),
            "onesc": to_f16(np.ones((128, DK), np.float32)),
            "cosT": cosT,
            "sinT": sinT,
        })
    return in_maps


# ---------------------------------------------------------------------------
# device program
# ---------------------------------------------------------------------------

def build_program(with_bias=False):
    from concourse import bacc, library_config
    nc = bacc.Bacc("TRN2", debug=False)

    xT = nc.declare_dram_parameter("xT", [D, S], MMDT, isOutput=False).ap()
    wq = nc.declare_dram_parameter("wq", [D, NCOLS], MMDT, isOutput=False).ap()
    wk = nc.declare_dram_parameter("wk", [D, NCOLS], MMDT, isOutput=False).ap()
    wv = nc.declare_dram_parameter("wv", [D, NCOLS], MMDT, isOutput=False).ap()
    wo = nc.declare_dram_parameter("wo", [128, 2, D], MMDT, isOutput=False).ap()
    bq = nc.declare_dram_parameter("bq", [1, NCOLS], MMDT, isOutput=False).ap()
    bk = nc.declare_dram_parameter("bk", [1, NCOLS], MMDT, isOutput=False).ap()
    bv = nc.declare_dram_parameter("bv", [1, NCOLS], MMDT, isOutput=False).ap()
    ones_row_d = nc.declare_dram_parameter("ones_row", [1, 512], MMDT, isOutput=False).ap()
    onesc_d = nc.declare_dram_parameter("onesc", [128, DK], MMDT, isOutput=False).ap()
    cosT = nc.declare_dram_parameter("cosT", [128, S], F32, isOutput=False).ap()
    sinT = nc.declare_dram_parameter("sinT", [128, S], F32, isOutput=False).ap()
    out = nc.declare_dram_parameter("out", [S, D], F16, isOutput=True).ap()

    SB = 512            # sq block width
    NSB = S // SB       # 4
    NST = S // 128      # 16 key tiles / V tiles
    NDC = D // 128      # 8 contraction chunks
    GW = 2              # key tiles per score-psum group
    AUGW = DK + 8       # V head stride (72): 16B-aligned fp16 lhsT starts

    with tile.TileContext(nc) as tc, ExitStack() as ctx:
        nc.gpsimd.load_library(library_config.proxy)
        const = ctx.enter_context(tc.tile_pool(name="const", bufs=1))
        sbig = ctx.enter_context(tc.tile_pool(name="sbig", bufs=1))
        xts = ctx.enter_context(tc.tile_pool(name="xts", bufs=8))
        rtmp = ctx.enter_context(tc.tile_pool(name="rtmp", bufs=2))
        epool = ctx.enter_context(tc.tile_pool(name="epool", bufs=3))
        npool = ctx.enter_context(tc.tile_pool(name="npool", bufs=5))
        opool = ctx.enter_context(tc.tile_pool(name="opool", bufs=4))
        mm_ps = ctx.enter_context(tc.tile_pool(name="mm_ps", bufs=2, space="PSUM"))
        sc_ps = ctx.enter_context(tc.tile_pool(name="sc_ps", bufs=2, space="PSUM"))
        pv_ps = ctx.enter_context(tc.tile_pool(name="pv_ps", bufs=2, space="PSUM"))

        # --- constants / weights resident in SBUF (per-dc tiles: finer deps,
        # so the first projection matmuls start early)
        wq_sb = [const.tile([128, NCOLS], MMDT, tag=f"wq{dc}", name=f"wq{dc}")
                 for dc in range(NDC)]
        wk_sb = [const.tile([128, NCOLS], MMDT, tag=f"wk{dc}", name=f"wk{dc}")
                 for dc in range(NDC)]
        wv_sb = [const.tile([128, NCOLS], MMDT, tag=f"wv{dc}", name=f"wv{dc}")
                 for dc in range(NDC)]
        # DMA priority: wq/wk chunks feed the very first matmuls; xt chunks
        # for sb=0 are emitted inside proj(0); tables/wv/wo right after them.
        cos_sb = const.tile([128, S], F32, tag="cos")
        sin_sb = const.tile([128, S], F32, tag="sin")
        wo_sb = const.tile([128, 2, D], MMDT, tag="wo")

        def late_dmas():
            # wk on the sync queue right behind the interleaved wq/xt pairs;
            # big tables ride the (idle-at-start) DVE trigger queue in
            # parallel so rope/V inputs land early without delaying xt.
            for dc in range(NDC):
                nc.sync.dma_start(wk_sb[dc][:], wk[128 * dc:128 * dc + 128, :])
            nc.scalar.dma_start(cos_sb[:], cosT)
            nc.scalar.dma_start(sin_sb[:], sinT)
            for dc in range(NDC):
                nc.scalar.dma_start(wv_sb[dc][:], wv[128 * dc:128 * dc + 128, :])
            nc.scalar.dma_start(wo_sb[:], wo)
        if with_bias:
            bq_sb = const.tile([1, NCOLS], MMDT, tag="bq")
            bk_sb = const.tile([1, NCOLS], MMDT, tag="bk")
            bv_sb = const.tile([1, NCOLS], MMDT, tag="bv")
            nc.sync.dma_start(bq_sb[:], bq)
            nc.sync.dma_start(bk_sb[:], bk)
            nc.sync.dma_start(bv_sb[:], bv)
        ones_row = const.tile([1, SB], MMDT, tag="ones_row")
        nc.sync.dma_start(ones_row[:], ones_row_d)
        onesc_sb = const.tile([128, DK], MMDT, tag="onesc")
        nc.sync.dma_start(onesc_sb[:], onesc_d)

        # Q^T per (chunk, sq-block): chunk c holds heads {2c, 2c+1}
        qt = [[sbig.tile([128, SB], MMDT, tag=f"qt{c}_{sb}", name=f"qt{c}_{sb}")
               for sb in range(NSB)] for c in range(2)]
        # per-head K^T, zero-padded to 128 partitions (head data on its chunk
        # rows, the complementary 64 rows zeroed)
        kth = [[sbig.tile([128, SB], MMDT, tag=f"kh{h}_{sb}", name=f"kh{h}_{sb}")
                for sb in range(NSB)] for h in range(HG)]
        for h in range(HG):
            zrows = slice(DK, 128) if h % 2 == 0 else slice(0, DK)
            for sb in range(NSB):
                nc.vector.memset(kth[h][sb][zrows, :], 0.0)
        # V augmented with a ones column per head, per key tile.
        vaug = [sbig.tile([128, HG * AUGW], MMDT, tag=f"va{st}", name=f"va{st}")
                for st in range(NST)]
        # normalized O^T per (head-pair, sq-block): lanes 0:64 = even head's
        # dk, 64:128 = odd head's dk -- Wo contracts a dense K=128 per pair.
        ot = [[sbig.tile([128, SB], MMDT, tag=f"ot{p}_{j}", name=f"ot{p}_{j}")
               for j in range(NSB)] for p in range(2)]

        ncopy = [0]

        def out_copy(dst, src):
            # DVE only: an ACT Copy would thrash activation-table loads
            # against the exp/reciprocal tables (static func->table binding)
            nc.vector.tensor_copy(dst, src)

        def proj_units(sb, post_xt=None):
            """Emit the xt DMAs now; return 8 unit-closures (4 QK rope
            units + 4 V units) to be interleaved into the attention head
            loop so attention's DVE recips never queue behind rope work."""
            ss = slice(SB * sb, SB * sb + SB)
            xt_t = []
            for dc in range(NDC):
                if sb == 0:
                    # first phase: pair each wq chunk with its xt chunk so
                    # matmul dc can start as soon as pair dc lands
                    nc.sync.dma_start(wq_sb[dc][:], wq[128 * dc:128 * dc + 128, :])
                t = xts.tile([128, SB], MMDT, tag="xt", name=f"xt{sb}_{dc}")
                nc.sync.dma_start(t[:], xT[128 * dc:128 * dc + 128, ss])
                xt_t.append(t)
            if post_xt is not None:
                post_xt()

            def qk_unit(c, bname):
                ncol = slice(128 * c, 128 * c + 128)
                w_sb = wq_sb if bname == "bq" else wk_sb
                ps = mm_ps.tile([128, SB], F32, tag="mm")
                for dc in range(NDC):
                    nc.tensor.matmul(ps[:], w_sb[dc][:, ncol], xt_t[dc][:],
                                     start=(dc == 0),
                                     stop=(dc == NDC - 1 and not with_bias))
                if with_bias:
                    b_sb = bq_sb if bname == "bq" else bk_sb
                    nc.tensor.matmul(ps[:], b_sb[0:1, ncol], ones_row[0:1, :],
                                     start=False, stop=True)
                # rope: dst = ps*cos + shuffle(ps)*sin
                t_cos = rtmp.tile([128, SB], F32, tag="rc")
                nc.vector.tensor_mul(t_cos[:], ps[:], cos_sb[:, ss])
                t_shuf = rtmp.tile([128, SB], F32, tag="rs")
                nc.vector.stream_shuffle(t_shuf[:], ps[:], SHUF_MASK)
                t_sin = rtmp.tile([128, SB], F32, tag="rm")
                nc.gpsimd.tensor_mul(t_sin[:], t_shuf[:], sin_sb[:, ss])
                if bname == "bq":
                    nc.gpsimd.tensor_add(qt[c][sb][:], t_cos[:], t_sin[:])
                else:
                    nc.gpsimd.tensor_add(kth[2 * c][sb][0:DK, :],
                                         t_cos[0:DK, :], t_sin[0:DK, :])
                    nc.gpsimd.tensor_add(kth[2 * c + 1][sb][DK:128, :],
                                         t_cos[DK:128, :], t_sin[DK:128, :])

            def v_unit(st4):
                st = (SB // 128) * sb + st4
                ps = mm_ps.tile([128, SB], F32, tag="mm")
                for dc in range(NDC):
                    nc.tensor.matmul(ps[:, 0:NCOLS],
                                     xt_t[dc][:, 128 * st4:128 * st4 + 128],
                                     wv_sb[dc][:],
                                     start=(dc == 0),
                                     stop=(dc == NDC - 1 and not with_bias))
                if with_bias:
                    nc.tensor.matmul(ps[:, 0:NCOLS], ones_row[0:1, 0:128],
                                     bv_sb[0:1, :], start=False, stop=True)
                va = vaug[st][:].rearrange("p (h e) -> p h e", h=HG)
                nc.vector.tensor_copy(va[:, :, 0:DK],
                                      ps[:, 0:NCOLS].rearrange("p (h k) -> p h k", h=HG))
                nc.vector.tensor_copy(va[:, :, DK], onesc_sb[:, 0:HG])

            units = [lambda c=c, b=b: qk_unit(c, b)
                     for c in range(2) for b in ("bq", "bk")]
            units += [lambda st4=st4: v_unit(st4) for st4 in range(SB // 128)]
            return units

        def proj(sb, post_xt=None):
            us = proj_units(sb, post_xt=post_xt)
            us[0]()
            finish_norm()
            for u in us[1:]:
                u()

        def raw_act(out_ap, in_ap, func, scale=1.0):
            eng = nc.scalar
            inputs = [eng.lower_ap(in_ap),
                      mybir.ImmediateValue(dtype=mybir.dt.float32, value=0.0),
                      mybir.ImmediateValue(dtype=mybir.dt.float32, value=scale),
                      mybir.ImmediateValue(dtype=mybir.dt.float32, value=0.0)]
            return eng.add_instruction(mybir.InstActivation(
                name=eng.bass.get_next_instruction_name(),
                func=func, ins=inputs, outs=[eng.lower_ap(out_ap)]))

        pend_norm = []

        def finish_one_norm():
            jj, h, pv16, rec16 = pend_norm.pop(0)
            p, u = divmod(h, 2)
            bcp = mm_ps.tile([128, SB], F32, tag="mm")
            nc.tensor.matmul(bcp[0:DK, :], onesc_sb[64:65, :],
                             rec16[DK:DK + 1, :],
                             start=True, stop=True)
            bc = npool.tile([128, SB], MMDT, tag="bc")
            nc.vector.tensor_copy(bc[0:DK, :], bcp[0:DK, :])
            # all-SBUF fp16 multiply: DVE 2x/4x fast path
            nc.vector.tensor_mul(ot[p][jj][DK * u:DK * u + DK, :],
                                 pv16[0:DK, :], bc[0:DK, :])

        def finish_norm():
            while pend_norm:
                finish_one_norm()

        def attn(j, units=()):
            # S^T layout: psum group = GW key tiles x one sq block; exp on ACT
            # over the causally-valid column ranges only; PV accumulates
            # (V | ones) so row 64 is the softmax denominator. `units` are
            # next-projection closures interleaved two-per-head so their rope
            # work lands *behind* this phase's recips in the DVE queue.
            units = list(units)
            finish_norm()
            local_norms = []
            for h in range(HG):
                c, half = divmod(h, 2)
                pv = pv_ps.tile([128, SB], F32, tag="pv")
                ngrp = (4 * j + 4) // GW
                for g in range(ngrp):
                    sc = sc_ps.tile([128, GW * SB], F32, tag="sc")
                    # lo[t]: first causally-valid query column for key tile
                    # GW*g + t; scores/exp/PV all skip cols below it.
                    los = [min(max(128 * (GW * g + t - 4 * j), 0), SB)
                           for t in range(GW)]
                    for t in range(GW):
                        i = GW * g + t
                        lo = los[t]
                        if lo >= SB:
                            continue
                        nc.tensor.matmul(
                            sc[:, SB * t + lo:SB * t + SB],
                            kth[h][i // 4][:, 128 * (i % 4):128 * (i % 4) + 128],
                            qt[c][j][:, lo:SB],
                            start=True, stop=True)
                    e = epool.tile([128, GW * SB], MMDT, tag="e")
                    diag = GW * g + GW - 4 * j > 0
                    if not diag:
                        # one big exp call: ACT per-call overhead ~0.2us
                        nc.scalar.activation(e[:], sc[:],
                                             mybir.ActivationFunctionType.Exp,
                                             scale=SCALE)
                    else:
                        for t in range(GW):
                            lo = los[t]
                            if lo > 0:
                                # zero the causally-dead prefix: PV streams
                                # the full e width (uniform psum region)
                                nc.gpsimd.memset(e[:, SB * t:SB * t + lo], 0.0)
                            nc.scalar.activation(
                                e[:, SB * t + lo:SB * t + SB],
                                sc[:, SB * t + lo:SB * t + SB],
                                mybir.ActivationFunctionType.Exp,
                                scale=SCALE)
                            if 128 * (GW * g + t - 4 * j) >= 0:
                                # triangle band: zero e where query < key
                                band = e[:, SB * t + lo:SB * t + lo + 128]
                                nc.gpsimd.affine_select(
                                    out=band.rearrange("p (o f) -> p o f", o=1),
                                    in_=band.rearrange("p (o f) -> p o f", o=1),
                                    compare_op=mybir.AluOpType.is_ge,
                                    fill=0.0, base=0,
                                    pattern=[[-128, 1], [1, 128]],
                                    channel_multiplier=-1)
                    for t in range(GW):
                        i = GW * g + t
                        lhs = vaug[i][:].rearrange("p (h e) -> p h e", h=HG)[:, h, 0:DK + 1]
                        nc.tensor.matmul(
                            pv[0:DK + 1, :], lhs, e[:, SB * t:SB * t + SB],
                            start=(g == 0 and t == 0),
                            stop=(g == ngrp - 1 and t == GW - 1))
                # normalize part 1: copy pv to fp16 SBUF right away (frees
                # the psum ring slot; the mul later runs all-SBUF-fp16)
                pv16 = npool.tile([128, SB], MMDT, tag="pv16")
                nc.vector.tensor_copy(pv16[0:DK + 1, :], pv[0:DK + 1, :])
                local_norms.append((h, pv16))
            # normalize part 2: all 4 heads' reciprocals batched on ACT
            # (raw Reciprocal, ~1e-5 accurate) -> one activation-table load
            # here + one exp reload next phase, instead of per-head DVE
            # reciprocals whose 3.4us true latency the scheduler mis-prices
            # and turns into PE stalls. Broadcast+scale flush next phase.
            for h, pv16 in local_norms:
                rec16 = npool.tile([128, SB], MMDT, tag="rec16")
                raw_act(rec16[DK:DK + 1, :], pv16[DK:DK + 1, :],
                        mybir.ActivationFunctionType.Reciprocal)
                pend_norm.append((j, h, pv16, rec16))
                for u in units[2 * h:2 * h + 2]:
                    u()

        def wo_phase(jb):
            for st4 in range(4):
                if st4 == 1:
                    finish_norm()
                st = 4 * jb + st4
                rq = slice(128 * st4, 128 * st4 + 128)
                for dc in range(2):
                    cols = slice(SB * dc, SB * dc + SB)
                    ps = mm_ps.tile([128, SB], F32, tag="mm")
                    for p in range(2):
                        nc.tensor.matmul(ps[:], ot[p][jb][:, rq], wo_sb[:, p, cols],
                                         start=(p == 0), stop=(p == 1))
                    o_sb = opool.tile([128, SB], F16, tag="osb")
                    out_copy(o_sb[:], ps[:])
                    nc.sync.dma_start(out[128 * st:128 * st + 128, cols], o_sb[:])

        # phase schedule: PE stream stays ~2 phases ahead of its producers
        proj(0, post_xt=late_dmas)
        proj(1)
        attn(0)
        proj(2)
        wo_phase(0)
        attn(1)
        proj(3)
        attn(2)
        wo_phase(1)
        attn(3)
        wo_phase(2)
        wo_phase(3)

    nc.compile()
    return nc


_CACHED_NC = {}


def _get_program(with_bias=False):
    if with_bias not in _CACHED_NC:
        _CACHED_NC[with_bias] = build_program(with_bias=with_bias)
    return _CACHED_NC[with_bias]


# ---------------------------------------------------------------------------
# entry point
# ---------------------------------------------------------------------------

def kernel(x, token_position, Wq, bq, Wk, bk, Wv, bv, Wo, bo, _results=None):
    from concourse.bass_utils import run_bass_kernel_spmd

    in_maps = make_core_inputs(x, token_position, Wq, bq, Wk, bk, Wv, bv, Wo, bo)
    if _results is None:
        with_bias = any(float(np.abs(np.asarray(v)).max()) != 0.0
                        for v in (bq, bk, bv))
        nc = _get_program(with_bias=with_bias)
        res = run_bass_kernel_spmd(nc, in_maps, list(range(N_CORES)))
        _results = [res.results[i]["out"] for i in range(N_CORES)]
    bo = np.asarray(bo, dtype=np.float32)
    out = np.empty((B, S, D), dtype=np.float32)
    for b in range(B):
        acc = np.asarray(_results[HG * b], dtype=np.float32)
        for hg in range(1, HG):
            acc = acc + np.asarray(_results[HG * b + hg], dtype=np.float32)
        out[b] = acc + bo[None, :]
    return out
